# revision 7
# baseline (speedup 1.0000x reference)
"""ALIF spike + delay-buffer gather kernel for 8 TRN2 NeuronCores.

Problem (shapes hardcoded):
    V, threshold: (128, 32768) f32
    alpha, amplitude: (32768,) f32
    buffer: (16, 128, 32768) f32
    delays: (8,) int, delays_xarea: (4,) int  (values in [0, 16))
Output: (14, 128, 32768) f32 =
    [X, new_buffer[delays], new_buffer[delays_xarea], new_threshold]
where X = (V - (threshold+1) >= 0), new_threshold = threshold*alpha + X*amplitude,
new_buffer = [X, buffer[0], ..., buffer[14]].

Strategy: shard the neuron axis N=32768 across 8 cores (4096 cols each).
All ops are elementwise in (B, N) or row-copies along the leading delay
axis, so there is no cross-core communication.  The 12 delay indices are
read on the host and baked into the compiled graph as DMA routing:
 - output row with delay d == 0  <- X tile (computed in SBUF)
 - output row with delay d  > 0  <- buffer[d-1], copied DRAM->DRAM
 - buffer rows feeding multiple output rows are staged once through SBUF
   so HBM is read only once per distinct row.
"""

import numpy as np

from concourse import bass, mybir
from concourse.bass_utils import run_bass_kernel_spmd


def _ensure_ntff_hook():
    """Provide antenv.axon_hooks if the image lacks it, so
    run_bass_kernel_spmd(trace=True) can capture NTFF profiles via the
    axon plugin's C ABI instead of crashing on the import."""
    try:
        from antenv.axon_hooks import get_axon_ntff_profile_hook  # noqa: F401
        return
    except ImportError:
        pass
    import sys
    import types
    import ctypes
    import contextlib

    def _make_hook():
        so_path = "/opt/axon/libaxon_pjrt.so"
        try:
            lib = ctypes.CDLL(so_path)
        except OSError:
            return None
        if not hasattr(lib, "axon_start_nrt_profile"):
            return None
        lib.axon_start_nrt_profile.argtypes = [
            ctypes.POINTER(ctypes.c_int64), ctypes.c_size_t]
        lib.axon_start_nrt_profile.restype = ctypes.c_int64
        lib.axon_stop_nrt_profile.argtypes = [ctypes.c_char_p]
        lib.axon_stop_nrt_profile.restype = ctypes.c_int64

        @contextlib.contextmanager
        def _hook(output_dir, device_ids):
            import jax
            jax.devices()
            if device_ids:
                ids = (ctypes.c_int64 * len(device_ids))(*device_ids)
                rc = lib.axon_start_nrt_profile(ids, len(device_ids))
            else:
                rc = lib.axon_start_nrt_profile(None, 0)
            if rc != 0:
                raise RuntimeError(f"axon_start_nrt_profile rc={rc}")
            try:
                yield
            finally:
                n = lib.axon_stop_nrt_profile(str(output_dir).encode())
                if n < 0:
                    raise RuntimeError(f"axon_stop_nrt_profile rc={n}")

        return _hook

    hook = [None]
    mod = types.ModuleType("antenv.axon_hooks")

    def get_axon_ntff_profile_hook():
        if hook[0] is None:
            hook[0] = _make_hook()
        return hook[0]

    def set_axon_ntff_profile_hook(h):
        hook[0] = h

    mod.get_axon_ntff_profile_hook = get_axon_ntff_profile_hook
    mod.set_axon_ntff_profile_hook = set_axon_ntff_profile_hook
    import antenv
    antenv.axon_hooks = mod
    sys.modules["antenv.axon_hooks"] = mod


_ensure_ntff_hook()

N_CORES = 8
B = 128
N = 32768
DMAX = 16
ND = 8
NDX = 4
OUT_ROWS = 1 + ND + NDX + 1  # 14
COLS = N // N_CORES  # 4096 columns per core

# Max distinct multi-destination buffer rows staged through SBUF;
# beyond this they fall back to direct DRAM->DRAM copies (extra reads).
MAX_STAGE = 4

_F32 = mybir.dt.float32

# (delay pattern, cols) -> (nc, used_rows)
_cache: dict = {}

# Set by run when trace=True was requested (via BASS_TRACE env or _trace flag)
last_result = None


def _build(delays_all: tuple, cols: int):
    """Build the SPMD Bass graph for one core (identical on all cores)."""
    x_rows = [0] + [1 + i for i, d in enumerate(delays_all) if d == 0]
    used = sorted({d - 1 for d in delays_all if d > 0})
    dests: dict[int, list] = {u: [] for u in used}
    for i, d in enumerate(delays_all):
        if d > 0:
            dests[d - 1].append(1 + i)

    # Buffer rows with >1 destination get SBUF staging (dedupes the HBM read)
    multi = [u for u in used if len(dests[u]) > 1][:MAX_STAGE]
    single = [u for u in used if u not in multi]

    nc = bass.Bass()
    v = nc.declare_dram_parameter("V", [B, cols], _F32, isOutput=False)
    th = nc.declare_dram_parameter("threshold", [B, cols], _F32, isOutput=False)
    al = nc.declare_dram_parameter("alpha", [cols], _F32, isOutput=False)
    am = nc.declare_dram_parameter("amplitude", [cols], _F32, isOutput=False)
    if used:
        bp = nc.declare_dram_parameter("bufpack", [len(used), B, cols], _F32,
                                       isOutput=False)
    out = nc.declare_dram_parameter("out", [OUT_ROWS, B, cols], _F32,
                                    isOutput=True)

    n_out_dma = len(x_rows) + 1 + sum(len(dests[u]) for u in used)

    from contextlib import ExitStack
    with ExitStack() as ctx:
        vt = ctx.enter_context(nc.sbuf_tensor([B, cols], _F32))
        tt = ctx.enter_context(nc.sbuf_tensor([B, cols], _F32))
        xt = ctx.enter_context(nc.sbuf_tensor([B, cols], _F32))
        nt = ctx.enter_context(nc.sbuf_tensor([B, cols], _F32))
        ab = ctx.enter_context(nc.sbuf_tensor([B, cols], _F32))
        mb = ctx.enter_context(nc.sbuf_tensor([B, cols], _F32))
        stage = [ctx.enter_context(nc.sbuf_tensor(f"stage{k}", [B, cols], _F32))
                 for k in range(len(multi))]
        dma_in = ctx.enter_context(nc.semaphore("dma_in"))
        stg_sem = ctx.enter_context(nc.semaphore("stg_sem"))
        c_sem = ctx.enter_context(nc.semaphore("c_sem"))
        dma_out = ctx.enter_context(nc.semaphore("dma_out"))
        block = ctx.enter_context(nc.Block())

        @block.sync
        def _(sync):
            sync.dma_start(out=vt[:], in_=v[:]).then_inc(dma_in, 16)
            sync.dma_start(out=tt[:], in_=th[:]).then_inc(dma_in, 16)
            sync.dma_start(
                out=ab[:], in_=al[None, :].broadcast_to((B, cols))
            ).then_inc(dma_in, 16)
            sync.dma_start(
                out=mb[:], in_=am[None, :].broadcast_to((B, cols))
            ).then_inc(dma_in, 16)
            sync.wait_ge(c_sem, 2)
            for r in x_rows:
                sync.dma_start(out=out[r], in_=xt[:]).then_inc(dma_out, 16)
            sync.wait_ge(c_sem, 5)
            sync.dma_start(out=out[OUT_ROWS - 1], in_=nt[:]).then_inc(dma_out, 16)
            # Drain: every output byte landed before the NEFF retires.
            sync.wait_ge(dma_out, 16 * n_out_dma)

        @block.scalar
        def _(scalar):
            # Stage multi-destination rows through SBUF: one HBM read each.
            for k, u in enumerate(multi):
                scalar.dma_start(out=stage[k][:], in_=bp[used.index(u)]).then_inc(
                    stg_sem, 16)
            # Unique rows: direct DRAM->DRAM.
            for u in single:
                j = used.index(u)
                for r in dests[u]:
                    scalar.dma_start(out=out[r], in_=bp[j]).then_inc(dma_out, 16)
            if multi:
                scalar.wait_ge(stg_sem, 16 * len(multi))
            for k, u in enumerate(multi):
                for r in dests[u]:
                    scalar.dma_start(out=out[r], in_=stage[k][:]).then_inc(
                        dma_out, 16)

        @block.vector
        def _(vector):
            vector.wait_ge(dma_in, 64)
            # y = (threshold + 1.0) - V;  X = (y <= 0)
            # bit-exact mirror of reference's (V - (threshold+1.0) >= 0)
            vector.scalar_tensor_tensor(
                out=xt[:], in0=tt[:], scalar=1.0, in1=vt[:],
                op0=mybir.AluOpType.add,
                op1=mybir.AluOpType.subtract).then_inc(c_sem, 1)
            vector.wait_ge(c_sem, 1)
            vector.tensor_scalar(
                out=xt[:], in0=xt[:], scalar1=0.0, scalar2=None,
                op0=mybir.AluOpType.is_le).then_inc(c_sem, 1)
            # new_threshold = threshold*alpha + X*amplitude
            vector.tensor_tensor(out=nt[:], in0=tt[:], in1=ab[:],
                                 op=mybir.AluOpType.mult).then_inc(c_sem, 1)
            vector.wait_ge(c_sem, 2)
            vector.tensor_tensor(out=vt[:], in0=xt[:], in1=mb[:],
                                 op=mybir.AluOpType.mult).then_inc(c_sem, 1)
            vector.wait_ge(c_sem, 4)
            vector.tensor_tensor(out=nt[:], in0=nt[:], in1=vt[:],
                                 op=mybir.AluOpType.add).then_inc(c_sem, 1)

    return nc, used


def _shard_inputs(V, threshold, alpha, amplitude, buffer, used, cols):
    in_maps = []
    used_idx = np.asarray(used, dtype=np.int64)
    for c in range(N_CORES):
        sl = slice(c * cols, (c + 1) * cols)
        m = {
            "V": np.ascontiguousarray(V[:, sl]),
            "threshold": np.ascontiguousarray(threshold[:, sl]),
            "alpha": np.ascontiguousarray(alpha[sl]),
            "amplitude": np.ascontiguousarray(amplitude[sl]),
        }
        if used:
            m["bufpack"] = np.ascontiguousarray(buffer[used_idx][:, :, sl])
        in_maps.append(m)
    return in_maps


def kernel(V, threshold, alpha, amplitude, buffer, delays, delays_xarea,
           _trace=False):
    global last_result
    V = np.ascontiguousarray(np.asarray(V, dtype=np.float32))
    threshold = np.ascontiguousarray(np.asarray(threshold, dtype=np.float32))
    alpha = np.ascontiguousarray(np.asarray(alpha, dtype=np.float32))
    amplitude = np.ascontiguousarray(np.asarray(amplitude, dtype=np.float32))
    buffer = np.ascontiguousarray(np.asarray(buffer, dtype=np.float32))
    delays_all = tuple(int(d) for d in np.asarray(delays).reshape(-1)) + \
        tuple(int(d) for d in np.asarray(delays_xarea).reshape(-1))
    assert len(delays_all) == ND + NDX
    assert all(0 <= d < DMAX for d in delays_all)

    key = (delays_all, COLS)
    if key not in _cache:
        _cache[key] = _build(delays_all, COLS)
    nc, used = _cache[key]

    in_maps = _shard_inputs(V, threshold, alpha, amplitude, buffer, used, COLS)
    res = run_bass_kernel_spmd(nc, in_maps, list(range(N_CORES)),
                               trace=_trace)
    last_result = res

    out = np.empty((OUT_ROWS, B, N), dtype=np.float32)
    for c in range(N_CORES):
        out[:, :, c * COLS:(c + 1) * COLS] = res.results[c]["out"]
    return out
